# revision 2
# baseline (speedup 1.0000x reference)
"""Trainium2 Bass kernel for nn_MultiHeadMLP — v2 (fp8 DoubleRow + folded norms).

Sharding: data-parallel over the 4096-token sequence across 8 NeuronCores
(512 rows each); keys/values/projections replicated. No collectives.

Per-core dataflow:
  host packs: xT8/Wq8/kT8 as fp8e4 (DoubleRow layouts), v16/Wo16 bf16
  qT[64,2,s]   = Wq8^T x8            (fp8 DoubleRow matmuls, 0.5 cyc/col)
  qhat8        = qT * rsqrt(ssq_q)   (ssq via fp8-DR ones matmul; DVE mult)
  ssqT_k[k,t]  = ones^T sq_k         (tiny [128,1]-free matmuls, k on partitions)
  rstd_colT    = |scale|/sqrt(ssq+eps)  (one small ACT op per head)
  att[k,s]     = kT8^T qhat8         (fp8 DoubleRow)
  E[k,s]       = exp(rstd[k]*att)    (ACT per-partition scale — key norm folded in)
  y            = v16^T E             (bf16 matmuls)
  den          = tree-add of E tiles (DVE bf16)
  ynorm        = y * recip(den)      (DVE)
  out16[s,o]   = ynorm^T Wo16        (bf16 matmuls, bf16 output DMA)
"""
import numpy as np
import ml_dtypes

import concourse.bacc as bacc
import concourse.mybir as mybir
import concourse.tile as tile
from concourse.bass_utils import run_bass_kernel_spmd

B, S, D = 1, 4096, 1024
H, HD, K = 8, 128, 2048
EPS = 1e-6
N_CORES = 8
SC = S // N_CORES      # 512 sequence rows per core
KT = K // 128          # 16 key tiles per head
MT = D // 128          # 8 contraction tiles for D
f32 = mybir.dt.float32
bf16 = mybir.dt.bfloat16
f8 = mybir.dt.float8e4
AF = mybir.ActivationFunctionType
OP = mybir.AluOpType
DR = mybir.MatmulPerfMode.DoubleRow


def build_nc(neg_heads=(), reps=1):
    import concourse.bass as bass

    nc = bacc.Bacc("TRN2", target_bir_lowering=False, debug=False, num_devices=N_CORES)
    xT8 = nc.dram_tensor("xT8", [128, MT, SC], f8, kind="ExternalInput").ap()
    Wq8 = nc.dram_tensor("Wq8", [128, MT, D], f8, kind="ExternalInput").ap()
    kT8 = nc.dram_tensor("kT8", [128, H, K], f8, kind="ExternalInput").ap()
    v16 = nc.dram_tensor("v16", [128, H, KT, HD], bf16, kind="ExternalInput").ap()
    Wo16 = nc.dram_tensor("Wo16", [128, H, D], bf16, kind="ExternalInput").ap()
    scale = nc.dram_tensor("scale", [H], f32, kind="ExternalInput").ap()
    out = nc.dram_tensor("out", [SC, D], bf16, kind="ExternalOutput").ap()

    with tile.TileContext(nc) as tc:
        def body():
            with tc.tile_pool(name="consts", bufs=1) as consts, \
                 tc.tile_pool(name="kts_p", bufs=1) as kts_p, \
                 tc.tile_pool(name="qhat_p", bufs=1) as qhat_p, \
                 tc.tile_pool(name="rstd_p", bufs=1) as rstd_p, \
                 tc.tile_pool(name="ynorm_p", bufs=1) as ynorm_p:

                # ---- constants
                eps_t = consts.tile([128, 1], f32)
                nc.vector.memset(eps_t[:], EPS)
                ones8 = consts.tile([64, 2, 64], f8)
                nc.vector.memset(ones8[:], 1.0)
                ones16 = consts.tile([128, 128], bf16)
                nc.vector.memset(ones16[:], 1.0)
                # w1col = 1/scale^2 per head, broadcast on all partitions (bf16)
                sc_sb = consts.tile([128, H], f32)
                sc_bcast = bass.AP(tensor=scale.tensor, offset=scale.offset,
                                   ap=[[0, 128], [1, H]])
                nc.gpsimd.dma_start(out=sc_sb[:], in_=sc_bcast)
                rs = consts.tile([128, H], f32)
                nc.vector.reciprocal(out=rs[:], in_=sc_sb[:])
                w1col = consts.tile([128, H], bf16)
                nc.vector.tensor_tensor(out=w1col[:], in0=rs[:], in1=rs[:], op=OP.mult)

                # ---- persistent activations
                kT8p = kts_p.tile([64, 2, H, K], f8)       # 2MB packed keysT (d split 2x64)
                kT8f = kts_p.tile([128, H, K], f8)         # 2MB flat keysT (for squares)
                qhat8 = qhat_p.tile([64, 2, H, SC], f8)    # .5MB normalized queriesT packed
                rstd_colT = rstd_p.tile([128, H, KT], f32) # per-slot |scale|/||k||, k on parts
                ynorm = ynorm_p.tile([128, H, SC], bf16)   # 1MB attention outputT

                # ---- Phase A: loads, key norms (transposed), q proj+norm
                with tc.tile_pool(name="wq_p", bufs=1) as wq_p, \
                     tc.tile_pool(name="xt_p", bufs=1) as xt_p, \
                     tc.tile_pool(name="sqk_p", bufs=2) as sqk_p, \
                     tc.tile_pool(name="sqq_p", bufs=2) as sqq_p, \
                     tc.tile_pool(name="rq_p", bufs=2) as rq_p, \
                     tc.tile_pool(name="ps_ssk", bufs=2, space="PSUM") as ps_ssk, \
                     tc.tile_pool(name="ps_qt", bufs=2, space="PSUM") as ps_qt, \
                     tc.tile_pool(name="ps_sq", bufs=2, space="PSUM") as ps_sq:
                    Wq_sb = wq_p.tile([128, MT, D], f8)
                    xT_sb = xt_p.tile([128, MT, SC], f8)
                    nc.sync.dma_start(out=xT_sb[:], in_=xT8[:])
                    nc.sync.dma_start(out=Wq_sb[:], in_=Wq8[:])
                    nc.gpsimd.dma_start(out=kT8f[:], in_=kT8[:])
                    kT8_packed_src = kT8.rearrange("(i p) h k -> p i h k", i=2)
                    nc.gpsimd.dma_start(out=kT8p[:], in_=kT8_packed_src)

                    # key norms: squares on gpsimd, ssqT via tiny matmuls
                    for h in range(H):
                        sq16 = sqk_p.tile([128, K], bf16, tag="sqk")
                        nc.gpsimd.tensor_tensor(out=sq16[:], in0=kT8f[:, h, :],
                                                in1=kT8f[:, h, :], op=OP.mult)
                        ssqT_ps = ps_ssk.tile([128, KT], f32, tag="ssk")
                        for t in range(KT):
                            # one 2KB zero-region: only t==0 may start
                            nc.tensor.matmul(ssqT_ps[:, t:t + 1],
                                             sq16[:, t * 128:(t + 1) * 128],
                                             w1col[:, h:h + 1], start=(t == 0),
                                             stop=True, skip_group_check=True)
                        nc.scalar.activation(out=rstd_colT[:, h, :], in_=ssqT_ps[:],
                                             func=AF.Abs_reciprocal_sqrt,
                                             bias=eps_t[:], scale=1.0)

                    # q projection (fp8 DoubleRow), then normalize
                    for h in range(H):
                        qt_ps = ps_qt.tile([64, 2, SC], f32, tag="qt")
                        for i in range(2):
                            for c in range(SC // 256):
                                for j in range(MT // 2):
                                    # qt_ps row i spans one 2KB region covering
                                    # both c chunks: start only at (c==0, j==0)
                                    nc.tensor.matmul(
                                        qt_ps[:, i, c * 256:(c + 1) * 256],
                                        Wq_sb[:, 2 * j:2 * j + 2,
                                              h * 128 + i * 64:h * 128 + i * 64 + 64],
                                        xT_sb[:, 2 * j:2 * j + 2, c * 256:(c + 1) * 256],
                                        start=(j == 0 and c == 0),
                                        stop=(j == MT // 2 - 1),
                                        perf_mode=DR, skip_group_check=True)
                        sq8q = sqq_p.tile([64, 2, SC], f8, tag="sqq")
                        nc.scalar.activation(out=sq8q[:], in_=qt_ps[:],
                                             func=AF.Square, bias=0.0, scale=1.0)
                        ssq_ps = ps_sq.tile([64, SC], f32, tag="ssq")
                        for c in range(SC // 256):
                            nc.tensor.matmul(ssq_ps[:, c * 256:(c + 1) * 256],
                                             ones8[:],
                                             sq8q[:, :, c * 256:(c + 1) * 256],
                                             start=(c == 0), stop=True,
                                             perf_mode=DR, skip_group_check=True)
                        rq = rq_p.tile([64, SC], f32, tag="rq")
                        nc.scalar.activation(out=rq[:], in_=ssq_ps[:],
                                             func=AF.Abs_reciprocal_sqrt,
                                             bias=eps_t[0:64, :], scale=1.0)
                        sgn = -1.0 if h in neg_heads else 1.0
                        for i in range(2):
                            nc.vector.scalar_tensor_tensor(
                                out=qhat8[:, i, h, :], in0=qt_ps[:, i, :],
                                scalar=sgn, in1=rq[:],
                                op0=OP.mult, op1=OP.mult)

                # ---- Phase B: attention per head
                with tc.tile_pool(name="vload", bufs=3) as vload, \
                     tc.tile_pool(name="e_p", bufs=2) as e_p, \
                     tc.tile_pool(name="dtree", bufs=2) as dtree, \
                     tc.tile_pool(name="rec_p", bufs=2) as rec_p, \
                     tc.tile_pool(name="ps_att", bufs=2, space="PSUM") as ps_att, \
                     tc.tile_pool(name="ps_y", bufs=2, space="PSUM") as ps_y, \
                     tc.tile_pool(name="ps_den", bufs=2, space="PSUM") as ps_den:
                    for h in range(H):
                        v_sb = vload.tile([128, KT, HD], bf16, tag="vf")
                        nc.sync.dma_start(out=v_sb[:], in_=v16[:, h, :, :])
                        E16 = e_p.tile([128, KT, SC], bf16, tag="E")
                        yt_ps = ps_y.tile([128, SC], f32, tag="yt")
                        for j in range(KT // 2):
                            att_ps = ps_att.tile([128, 2, SC], f32, tag="att")
                            for i in range(2):
                                t = 2 * j + i
                                for c in range(SC // 256):
                                    nc.tensor.matmul(
                                        att_ps[:, i, c * 256:(c + 1) * 256],
                                        kT8p[:, :, h, t * 128:(t + 1) * 128],
                                        qhat8[:, :, h, c * 256:(c + 1) * 256],
                                        start=(c == 0), stop=True,
                                        perf_mode=DR, skip_group_check=True)
                                nc.scalar.activation(out=E16[:, t, :], in_=att_ps[:, i, :],
                                                     func=AF.Exp, bias=0.0,
                                                     scale=rstd_colT[:, h, t:t + 1])
                                nc.tensor.matmul(yt_ps[:], v_sb[:, t, :], E16[:, t, :],
                                                 start=(t == 0), stop=(t == KT - 1))
                        # denominator: bf16 tree reduction on DVE
                        d1 = dtree.tile([128, 8, SC], bf16, tag="d1")
                        nc.vector.tensor_tensor(out=d1[:], in0=E16[:, 0:8, :],
                                                in1=E16[:, 8:16, :], op=OP.add)
                        d2 = dtree.tile([128, 4, SC], bf16, tag="d2")
                        nc.vector.tensor_tensor(out=d2[:], in0=d1[:, 0:4, :],
                                                in1=d1[:, 4:8, :], op=OP.add)
                        d3 = dtree.tile([128, 2, SC], bf16, tag="d3")
                        nc.vector.tensor_tensor(out=d3[:], in0=d2[:, 0:2, :],
                                                in1=d2[:, 2:4, :], op=OP.add)
                        d4 = dtree.tile([128, SC], bf16, tag="d4")
                        nc.vector.tensor_tensor(out=d4[:], in0=d3[:, 0, :],
                                                in1=d3[:, 1, :], op=OP.add)
                        # partition-axis sum of the tree result
                        den_ps = ps_den.tile([128, SC], f32, tag="den")
                        nc.tensor.matmul(den_ps[:], ones16[:], d4[:],
                                         start=True, stop=True)
                        recd = rec_p.tile([128, SC], f32, tag="recd")
                        nc.vector.reciprocal_approx_fast(out=recd[:], in_=den_ps[:])
                        nc.vector.tensor_tensor(out=ynorm[:, h, :], in0=yt_ps[:],
                                                in1=recd[:], op=OP.mult)

                # ---- Phase C: output projection (phase-B psum pools closed)
                with tc.tile_pool(name="wo_p", bufs=1) as wo_p, \
                     tc.tile_pool(name="outsb", bufs=3) as outsb, \
                     tc.tile_pool(name="ps_out", bufs=2, space="PSUM") as ps_out:
                    Wo_sb = wo_p.tile([128, H, D], bf16)
                    nc.sync.dma_start(out=Wo_sb[:], in_=Wo16[:])
                    for si in range(SC // 128):
                        for oc in range(D // 512):
                            o_ps = ps_out.tile([128, 512], f32, tag="ops")
                            for h in range(H):
                                nc.tensor.matmul(o_ps[:],
                                                 ynorm[:, h, si * 128:(si + 1) * 128],
                                                 Wo_sb[:, h, oc * 512:(oc + 1) * 512],
                                                 start=(h == 0), stop=(h == H - 1))
                            o_sb = outsb.tile([128, 512], bf16, tag="osb")
                            nc.vector.tensor_copy(out=o_sb[:], in_=o_ps[:])
                            nc.sync.dma_start(
                                out=out[si * 128:(si + 1) * 128,
                                        oc * 512:(oc + 1) * 512],
                                in_=o_sb[:])

        if reps > 1:
            with tc.For_i(0, reps, 1):
                body()
        else:
            body()

    nc.compile()
    return nc


_CACHE = {}


def _get_nc(neg_heads, reps=1):
    key = (tuple(sorted(neg_heads)), reps)
    if key not in _CACHE:
        _CACHE[key] = build_nc(neg_heads, reps)
    return _CACHE[key]


def _make_in_maps(x, Wq, keys, values, attn_scale, Wo):
    f8np = ml_dtypes.float8_e4m3
    bf16np = ml_dtypes.bfloat16
    x = np.asarray(x, dtype=np.float32).reshape(S, D)
    Wq = np.asarray(Wq, dtype=np.float32)
    Wo = np.asarray(Wo, dtype=np.float32)
    keys = np.asarray(keys, dtype=np.float32).reshape(K, H, HD)
    values = np.asarray(values, dtype=np.float32).reshape(K, H, HD)
    attn_scale = np.ascontiguousarray(np.asarray(attn_scale, dtype=np.float32))

    # Wq8 [128, MT, D]: (p, mt, n) = Wq[mt*128+p, n]
    Wq8 = np.ascontiguousarray(
        Wq.reshape(MT, 128, D).transpose(1, 0, 2)).astype(f8np)
    # kT8 [128, H, K]: (d, h, k) = keys[k, h, d]
    kT8 = np.ascontiguousarray(keys.transpose(2, 1, 0)).astype(f8np)
    # v16 [128, H, KT, HD]: (p, h, t, d) = values[t*128+p, h, d]
    v16 = np.ascontiguousarray(
        values.reshape(KT, 128, H, HD).transpose(1, 2, 0, 3)).astype(bf16np)
    # Wo16 [128, H, D]: (p, h, o) = Wo[h*128+p, o]
    Wo16 = np.ascontiguousarray(
        Wo.reshape(H, 128, D).transpose(1, 0, 2)).astype(bf16np)

    in_maps = []
    for c in range(N_CORES):
        xc = x[c * SC:(c + 1) * SC, :]                     # [SC, D]
        # xT8 [128, MT, SC]: (p, mt, s) = x[s, mt*128+p]
        xT8 = np.ascontiguousarray(
            xc.T.reshape(MT, 128, SC).transpose(1, 0, 2)).astype(f8np)
        in_maps.append({
            "xT8": xT8, "Wq8": Wq8, "kT8": kT8, "v16": v16, "Wo16": Wo16,
            "scale": attn_scale,
        })
    return in_maps


def kernel(x, Wq, keys, values, attn_scale, Wo):
    neg_heads = tuple(np.nonzero(np.asarray(attn_scale) < 0)[0].tolist())
    nc = _get_nc(neg_heads)
    in_maps = _make_in_maps(x, Wq, keys, values, attn_scale, Wo)
    res = run_bass_kernel_spmd(nc, in_maps, list(range(N_CORES)))
    out = np.concatenate([np.asarray(r["out"], dtype=np.float32)
                          for r in res.results], axis=0)
    return out.reshape(B, S, D)


# revision 4
# speedup vs baseline: 1.2340x; 1.2340x over previous
"""Trainium2 Bass kernel for nn_MultiHeadMLP — v5 (stage-major pipelining).

Measured per-instruction costs on this stack (PE mm ~110ns; ACT ~500ns fixed
+0.57ns/col; DVE 4x bf16 ~0.3ns/col; Pool ~1.5ns/col) drive the design:
  - all-bf16, flat layouts; keys normalized so exp takes no scale
  - stage-major emission: every engine queue holds same-stage ops of all
    heads back-to-back, so cross-engine chains pipeline instead of rippling
  - per-head DMA chunks so the first head's work starts ~2us in
  - phase B: exp in [128,1536] batches, att pool double-buffered (6 banks),
    y accumulators 2 banks; den fully on DVE tree; lag-one head norm
"""
import numpy as np
import ml_dtypes

import concourse.bacc as bacc
import concourse.mybir as mybir
import concourse.tile as tile
from concourse.bass_utils import run_bass_kernel_spmd

B, S, D = 1, 4096, 1024
H, HD, K = 8, 128, 2048
EPS = 1e-6
N_CORES = 8
SC = S // N_CORES
KT = K // 128
MT = D // 128
f32 = mybir.dt.float32
bf16 = mybir.dt.bfloat16
AF = mybir.ActivationFunctionType
OP = mybir.AluOpType


def build_nc(neg_heads=(), reps=1):
    import concourse.bass as bass

    nc = bacc.Bacc("TRN2", target_bir_lowering=False, debug=False, num_devices=N_CORES)
    xT16 = nc.dram_tensor("xT16", [128, MT, SC], bf16, kind="ExternalInput").ap()
    Wq16 = nc.dram_tensor("Wq16", [128, MT, D], bf16, kind="ExternalInput").ap()
    kT16 = nc.dram_tensor("kT16", [128, H, K], bf16, kind="ExternalInput").ap()
    v16 = nc.dram_tensor("v16", [128, H, KT, HD], bf16, kind="ExternalInput").ap()
    Wo16 = nc.dram_tensor("Wo16", [128, H, D], bf16, kind="ExternalInput").ap()
    scale = nc.dram_tensor("scale", [H], f32, kind="ExternalInput").ap()
    out = nc.dram_tensor("out", [SC, D], bf16, kind="ExternalOutput").ap()

    with tile.TileContext(nc) as tc:
        def body():
            with tc.tile_pool(name="consts", bufs=1) as consts, \
                 tc.tile_pool(name="kts_p", bufs=1) as kts_p, \
                 tc.tile_pool(name="qhat_p", bufs=1) as qhat_p, \
                 tc.tile_pool(name="ynorm_p", bufs=1) as ynorm_p:

                # ---- constants
                eps_t = consts.tile([128, 1], f32)
                nc.vector.memset(eps_t[:], EPS)
                ones16 = consts.tile([128, 128], bf16)
                nc.vector.memset(ones16[:], 1.0)
                sc_sb = consts.tile([128, H], f32)
                sc_bcast = bass.AP(tensor=scale.tensor, offset=scale.offset,
                                   ap=[[0, 128], [1, H]])
                nc.gpsimd.dma_start(out=sc_sb[:], in_=sc_bcast)
                rs = consts.tile([128, H], f32)
                nc.vector.reciprocal(out=rs[:], in_=sc_sb[:])
                rs2 = consts.tile([128, H], f32)
                nc.vector.tensor_tensor(out=rs2[:], in0=rs[:], in1=rs[:], op=OP.mult)
                w1m = consts.tile([128, H, 128], bf16)
                for h in range(H):
                    nc.vector.tensor_scalar(out=w1m[:, h, :], in0=ones16[:],
                                            scalar1=rs2[:, h:h + 1], scalar2=None,
                                            op0=OP.mult)

                # ---- persistent
                kts16 = kts_p.tile([128, H, K], bf16)
                qhat16 = qhat_p.tile([128, H, SC], bf16)
                ynorm = ynorm_p.tile([128, H, SC], bf16)

                # ---- Phase A (stage-major)
                with tc.tile_pool(name="wq_p", bufs=1) as wq_p, \
                     tc.tile_pool(name="xt_p", bufs=1) as xt_p, \
                     tc.tile_pool(name="ktf_p", bufs=1) as ktf_p, \
                     tc.tile_pool(name="sqk_p", bufs=1) as sqk_p, \
                     tc.tile_pool(name="rk_p", bufs=2) as rk_p, \
                     tc.tile_pool(name="qt16_p", bufs=1) as qt16_p, \
                     tc.tile_pool(name="sqq_p", bufs=2) as sqq_p, \
                     tc.tile_pool(name="rq_p", bufs=2) as rq_p, \
                     tc.tile_pool(name="ps_qt", bufs=2, space="PSUM") as ps_qt, \
                     tc.tile_pool(name="ps_sqq", bufs=1, space="PSUM") as ps_sqq, \
                     tc.tile_pool(name="ps_ssk", bufs=1, space="PSUM") as ps_ssk:
                    Wq_sb = wq_p.tile([128, MT, D], bf16)
                    xT_sb = xt_p.tile([128, MT, SC], bf16)
                    kT16f = ktf_p.tile([128, H, K], bf16)
                    sq16k = sqk_p.tile([128, H, K], bf16)
                    qt16 = qt16_p.tile([128, H, SC], bf16)

                    # DMAs: x first, then per-head Wq cols (sync queue);
                    # keys per-head (gpsimd queue)
                    nc.sync.dma_start(out=xT_sb[:], in_=xT16[:])
                    for h in range(H):
                        nc.sync.dma_start(
                            out=Wq_sb[:, :, h * 128:(h + 1) * 128],
                            in_=Wq16[:, :, h * 128:(h + 1) * 128])
                    for h in range(H):
                        nc.gpsimd.dma_start(out=kT16f[:, h, :], in_=kT16[:, h, :])

                    # S1: key squares on DVE (bf16 4x mode)
                    for h in range(H):
                        nc.vector.tensor_tensor(out=sq16k[:, h, :],
                                                in0=kT16f[:, h, :],
                                                in1=kT16f[:, h, :], op=OP.mult)

                    # S2: q projections (PE) + psum->sbuf copy (DVE)
                    qt_pss = {}
                    for h in range(H):
                        qt_ps = ps_qt.tile([128, SC], f32, tag="qt")
                        for m in range(MT):
                            nc.tensor.matmul(qt_ps[:],
                                             Wq_sb[:, m, h * 128:(h + 1) * 128],
                                             xT_sb[:, m, :],
                                             start=(m == 0), stop=(m == MT - 1))
                        nc.vector.tensor_copy(out=qt16[:, h, :], in_=qt_ps[:])
                        qt_pss[h] = qt_ps

                    # S3: keys ssq (PE; Pool squares done head-wise) + rstd
                    # (ACT) + kts (DVE); ssk single-buffered 4-bank tile
                    for h in range(H):
                        ssqk_ps = ps_ssk.tile([128, K], f32, tag="ssk")
                        for c in range(K // 512):
                            nc.tensor.matmul(ssqk_ps[:, c * 512:(c + 1) * 512],
                                             w1m[:, h, :],
                                             sq16k[:, h, c * 512:(c + 1) * 512],
                                             start=True, stop=True)
                        rstd16 = rk_p.tile([128, K], bf16, tag="rk")
                        nc.scalar.activation(out=rstd16[:], in_=ssqk_ps[:],
                                             func=AF.Abs_reciprocal_sqrt,
                                             bias=eps_t[:], scale=1.0)
                        nc.vector.tensor_tensor(out=kts16[:, h, :],
                                                in0=kT16f[:, h, :],
                                                in1=rstd16[:], op=OP.mult)

                    # S4: q squares (Pool) + ssq matmuls into head-pairs
                    # (PE) + rsqrt per pair (ACT) + qhat (DVE)
                    sqq_all = sqq_p.tile([128, H, SC], bf16, tag="sqq")
                    for h in range(H):
                        nc.gpsimd.tensor_tensor(out=sqq_all[:, h, :],
                                                in0=qt16[:, h, :],
                                                in1=qt16[:, h, :], op=OP.mult)
                    rqs = {}
                    for hp in range(H // 2):
                        ssqq_ps = ps_sqq.tile([128, 2, SC], f32, tag="ssqq")
                        for i in range(2):
                            nc.tensor.matmul(ssqq_ps[:, i, :], ones16[:],
                                             sqq_all[:, 2 * hp + i, :],
                                             start=True, stop=True)
                        rqp = rq_p.tile([128, 2, SC], f32, tag="rq")
                        nc.scalar.activation(out=rqp[:], in_=ssqq_ps[:],
                                             func=AF.Abs_reciprocal_sqrt,
                                             bias=eps_t[:], scale=1.0)
                        rqs[hp] = rqp
                    for h in range(H):
                        sgn = -1.0 if h in neg_heads else 1.0
                        nc.vector.scalar_tensor_tensor(
                            out=qhat16[:, h, :], in0=qt16[:, h, :], scalar=sgn,
                            in1=rqs[h // 2][:, h % 2, :], op0=OP.mult, op1=OP.mult)

                # ---- Phase B (att pool double-buffered, exp 3-ktile batches)
                BATCHES = [(0, 3), (3, 3), (6, 3), (9, 3), (12, 3), (15, 1)]
                with tc.tile_pool(name="vload", bufs=3) as vload, \
                     tc.tile_pool(name="e_p", bufs=2) as e_p, \
                     tc.tile_pool(name="d1_p", bufs=2) as d1_p, \
                     tc.tile_pool(name="d2_p", bufs=2) as d2_p, \
                     tc.tile_pool(name="rec_p", bufs=2) as rec_p, \
                     tc.tile_pool(name="wo_p", bufs=1) as wo_p:
                    Wo_sb = wo_p.tile([128, H, D], bf16)
                    hstate = {}
                    P = {}

                    def emit_head_fwd(h):
                        ps_att, ps_y = P["att"], P["y"]
                        v_sb = vload.tile([128, KT, HD], bf16, tag="vf")
                        nc.gpsimd.dma_start(out=v_sb[:], in_=v16[:, h, :, :])
                        E16 = e_p.tile([128, KT, SC], bf16, tag="E")
                        yt_ps = ps_y.tile([128, SC], f32, tag="yt")
                        prev = None
                        for t0, nt in BATCHES:
                            att_ps = ps_att.tile([128, 3, SC], f32, tag="att")
                            for tt in range(nt):
                                t = t0 + tt
                                nc.tensor.matmul(
                                    att_ps[:, tt, :],
                                    kts16[:, h, t * 128:(t + 1) * 128],
                                    qhat16[:, h, :], start=True, stop=True)
                            nc.scalar.activation(
                                out=E16[:, t0:t0 + nt, :],
                                in_=att_ps[:, 0:nt, :], func=AF.Exp,
                                bias=0.0, scale=1.0)
                            if prev is not None:   # y of previous batch
                                p0, pn = prev
                                for tt in range(pn):
                                    t = p0 + tt
                                    nc.tensor.matmul(yt_ps[:], v_sb[:, t, :],
                                                     E16[:, t, :],
                                                     start=(t == 0), stop=False)
                            prev = (t0, nt)
                        p0, pn = prev
                        for tt in range(pn):
                            t = p0 + tt
                            nc.tensor.matmul(yt_ps[:], v_sb[:, t, :], E16[:, t, :],
                                             start=False, stop=(t == KT - 1))
                        hstate[h] = (E16, yt_ps)

                    def emit_head_norm(h):
                        E16, yt_ps = hstate.pop(h)
                        d1 = d1_p.tile([128, 8, SC], bf16, tag="d1")
                        nc.vector.tensor_tensor(out=d1[:], in0=E16[:, 0:8, :],
                                                in1=E16[:, 8:16, :], op=OP.add)
                        d2 = d2_p.tile([128, 4, SC], bf16, tag="d2")
                        nc.vector.tensor_tensor(out=d2[:], in0=d1[:, 0:4, :],
                                                in1=d1[:, 4:8, :], op=OP.add)
                        # partition-axis reduction on PE (den borrows an
                        # att-ring slot so yt can double-buffer)
                        den_t = P["att"].tile([128, 3, SC], f32, tag="att")
                        den_ps = den_t[:, 0, :]
                        for g in range(4):
                            nc.tensor.matmul(den_ps, ones16[:], d2[:, g, :],
                                             start=(g == 0), stop=(g == 3))
                        recd = rec_p.tile([128, SC], f32, tag="recd")
                        nc.vector.reciprocal_approx_fast(out=recd[:], in_=den_ps)
                        nc.vector.tensor_tensor(out=ynorm[:, h, :], in0=yt_ps[:],
                                                in1=recd[:], op=OP.mult)

                    with tc.tile_pool(name="ps_att", bufs=2, space="PSUM") as ps_att, \
                         tc.tile_pool(name="ps_y", bufs=2, space="PSUM") as ps_y:
                        P["att"], P["y"] = ps_att, ps_y
                        for h in range(H):
                            emit_head_fwd(h)
                            if h == 6:
                                nc.sync.dma_start(out=Wo_sb[:], in_=Wo16[:])
                            if h >= 1:
                                emit_head_norm(h - 1)
                        emit_head_norm(H - 1)

                    # ---- Phase C
                    with tc.tile_pool(name="outsb", bufs=1) as outsb, \
                         tc.tile_pool(name="ps_out", bufs=2, space="PSUM") as ps_out:
                        out_all = outsb.tile([128, 4, 2, 512], bf16)
                        for si in range(SC // 128):
                            for oc in range(D // 512):
                                o_ps = ps_out.tile([128, 512], f32, tag="ops")
                                for h in range(H):
                                    nc.tensor.matmul(
                                        o_ps[:],
                                        ynorm[:, h, si * 128:(si + 1) * 128],
                                        Wo_sb[:, h, oc * 512:(oc + 1) * 512],
                                        start=(h == 0), stop=(h == H - 1))
                                nc.vector.tensor_copy(out=out_all[:, si, oc, :],
                                                      in_=o_ps[:])
                        out_view = out.rearrange("(si p) (oc c) -> p si oc c",
                                                 p=128, c=512)
                        nc.sync.dma_start(out=out_view, in_=out_all[:])

        if reps > 1:
            with tc.For_i(0, reps, 1):
                body()
        else:
            body()

    nc.compile()
    return nc


_CACHE = {}


def _get_nc(neg_heads, reps=1):
    key = (tuple(sorted(neg_heads)), reps)
    if key not in _CACHE:
        _CACHE[key] = build_nc(neg_heads, reps)
    return _CACHE[key]


def _make_in_maps(x, Wq, keys, values, attn_scale, Wo):
    bf16np = ml_dtypes.bfloat16
    x = np.asarray(x, dtype=np.float32).reshape(S, D)
    Wq = np.asarray(Wq, dtype=np.float32)
    Wo = np.asarray(Wo, dtype=np.float32)
    keys = np.asarray(keys, dtype=np.float32).reshape(K, H, HD)
    values = np.asarray(values, dtype=np.float32).reshape(K, H, HD)
    attn_scale = np.ascontiguousarray(np.asarray(attn_scale, dtype=np.float32))

    Wq16 = np.ascontiguousarray(
        Wq.reshape(MT, 128, D).transpose(1, 0, 2)).astype(bf16np)
    kT16 = np.ascontiguousarray(keys.transpose(2, 1, 0)).astype(bf16np)
    v16h = np.ascontiguousarray(
        values.reshape(KT, 128, H, HD).transpose(1, 2, 0, 3)).astype(bf16np)
    Wo16 = np.ascontiguousarray(
        Wo.reshape(H, 128, D).transpose(1, 0, 2)).astype(bf16np)

    in_maps = []
    for c in range(N_CORES):
        xc = x[c * SC:(c + 1) * SC, :]
        xT16c = np.ascontiguousarray(
            xc.T.reshape(MT, 128, SC).transpose(1, 0, 2)).astype(bf16np)
        in_maps.append({
            "xT16": xT16c, "Wq16": Wq16, "kT16": kT16, "v16": v16h,
            "Wo16": Wo16, "scale": attn_scale,
        })
    return in_maps


def kernel(x, Wq, keys, values, attn_scale, Wo):
    neg_heads = tuple(np.nonzero(np.asarray(attn_scale) < 0)[0].tolist())
    nc = _get_nc(neg_heads)
    in_maps = _make_in_maps(x, Wq, keys, values, attn_scale, Wo)
    res = run_bass_kernel_spmd(nc, in_maps, list(range(N_CORES)))
    out = np.concatenate([np.asarray(r["out"], dtype=np.float32)
                          for r in res.results], axis=0)
    return out.reshape(B, S, D)


# revision 5
# speedup vs baseline: 1.2894x; 1.0449x over previous
"""Trainium2 Bass kernel for nn_MultiHeadMLP — v5 (stage-major pipelining).

Measured per-instruction costs on this stack (PE mm ~110ns; ACT ~500ns fixed
+0.57ns/col; DVE 4x bf16 ~0.3ns/col; Pool ~1.5ns/col) drive the design:
  - all-bf16, flat layouts; keys normalized so exp takes no scale
  - stage-major emission: every engine queue holds same-stage ops of all
    heads back-to-back, so cross-engine chains pipeline instead of rippling
  - per-head DMA chunks so the first head's work starts ~2us in
  - phase B: exp in [128,1536] batches, att pool double-buffered (6 banks),
    y accumulators 2 banks; den fully on DVE tree; lag-one head norm
"""
import numpy as np
import ml_dtypes

import concourse.bacc as bacc
import concourse.bass_isa as bass_isa
import concourse.mybir as mybir
import concourse.tile as tile
from concourse.bass_utils import run_bass_kernel_spmd

B, S, D = 1, 4096, 1024
H, HD, K = 8, 128, 2048
EPS = 1e-6
N_CORES = 8
SC = S // N_CORES
KT = K // 128
MT = D // 128
f32 = mybir.dt.float32
bf16 = mybir.dt.bfloat16
AF = mybir.ActivationFunctionType
OP = mybir.AluOpType


def build_nc(neg_heads=(), reps=1):
    import concourse.bass as bass

    nc = bacc.Bacc("TRN2", target_bir_lowering=False, debug=False, num_devices=N_CORES)
    xT16 = nc.dram_tensor("xT16", [128, MT, SC], bf16, kind="ExternalInput").ap()
    Wq16 = nc.dram_tensor("Wq16", [128, MT, D], bf16, kind="ExternalInput").ap()
    kT16 = nc.dram_tensor("kT16", [128, H, K], bf16, kind="ExternalInput").ap()
    v16 = nc.dram_tensor("v16", [128, H, KT, HD], bf16, kind="ExternalInput").ap()
    Wo16 = nc.dram_tensor("Wo16", [128, H, D], bf16, kind="ExternalInput").ap()
    scale = nc.dram_tensor("scale", [H], f32, kind="ExternalInput").ap()
    out = nc.dram_tensor("out", [SC, D], bf16, kind="ExternalOutput").ap()

    with tile.TileContext(nc) as tc:
        def body():
            with tc.tile_pool(name="consts", bufs=1) as consts, \
                 tc.tile_pool(name="kts_p", bufs=1) as kts_p, \
                 tc.tile_pool(name="qhat_p", bufs=1) as qhat_p, \
                 tc.tile_pool(name="ynorm_p", bufs=1) as ynorm_p:

                # ---- constants
                eps_t = consts.tile([128, 1], f32)
                nc.vector.memset(eps_t[:], EPS)
                ones16 = consts.tile([128, 128], bf16)
                nc.vector.memset(ones16[:], 1.0)
                sc_sb = consts.tile([128, H], f32)
                sc_bcast = bass.AP(tensor=scale.tensor, offset=scale.offset,
                                   ap=[[0, 128], [1, H]])
                nc.gpsimd.dma_start(out=sc_sb[:], in_=sc_bcast)
                rs = consts.tile([128, H], f32)
                nc.vector.reciprocal(out=rs[:], in_=sc_sb[:])
                rs2 = consts.tile([128, H], f32)
                nc.vector.tensor_tensor(out=rs2[:], in0=rs[:], in1=rs[:], op=OP.mult)
                w1m = consts.tile([128, H, 128], bf16)
                for h in range(H):
                    nc.vector.tensor_scalar(out=w1m[:, h, :], in0=ones16[:],
                                            scalar1=rs2[:, h:h + 1], scalar2=None,
                                            op0=OP.mult)

                # ---- persistent
                kts16 = kts_p.tile([128, H, K], bf16)
                qhat16 = qhat_p.tile([128, H, SC], bf16)
                ynorm = ynorm_p.tile([128, H, SC], bf16)

                # ---- Phase A (stage-major)
                with tc.tile_pool(name="wq_p", bufs=1) as wq_p, \
                     tc.tile_pool(name="xt_p", bufs=1) as xt_p, \
                     tc.tile_pool(name="ktf_p", bufs=1) as ktf_p, \
                     tc.tile_pool(name="sqk_p", bufs=1) as sqk_p, \
                     tc.tile_pool(name="rk_p", bufs=2) as rk_p, \
                     tc.tile_pool(name="qt16_p", bufs=1) as qt16_p, \
                     tc.tile_pool(name="sqq_p", bufs=2) as sqq_p, \
                     tc.tile_pool(name="rq_p", bufs=2) as rq_p, \
                     tc.tile_pool(name="ps_qt", bufs=2, space="PSUM") as ps_qt, \
                     tc.tile_pool(name="ps_sqq", bufs=1, space="PSUM") as ps_sqq, \
                     tc.tile_pool(name="ps_ssk", bufs=1, space="PSUM") as ps_ssk:
                    Wq_sb = wq_p.tile([128, MT, D], bf16)
                    xT_sb = xt_p.tile([128, MT, SC], bf16)
                    kT16f = ktf_p.tile([128, H, K], bf16)
                    sq16k = sqk_p.tile([128, H, K], bf16)
                    qt16 = qt16_p.tile([128, H, SC], bf16)

                    # DMAs: x first, then per-head Wq cols (sync queue);
                    # keys per-head (gpsimd queue)
                    nc.sync.dma_start(out=xT_sb[:], in_=xT16[:])
                    for h in range(H):
                        nc.sync.dma_start(
                            out=Wq_sb[:, :, h * 128:(h + 1) * 128],
                            in_=Wq16[:, :, h * 128:(h + 1) * 128])
                    for h in range(H):
                        nc.gpsimd.dma_start(out=kT16f[:, h, :], in_=kT16[:, h, :])

                    # S1: key squares on DVE (bf16 4x mode)
                    for h in range(H):
                        nc.vector.tensor_tensor(out=sq16k[:, h, :],
                                                in0=kT16f[:, h, :],
                                                in1=kT16f[:, h, :], op=OP.mult)

                    # S2: q projections (PE) + psum->sbuf copy (DVE)
                    qt_pss = {}
                    for h in range(H):
                        qt_ps = ps_qt.tile([128, SC], f32, tag="qt")
                        for m in range(MT):
                            nc.tensor.matmul(qt_ps[:],
                                             Wq_sb[:, m, h * 128:(h + 1) * 128],
                                             xT_sb[:, m, :],
                                             start=(m == 0), stop=(m == MT - 1))
                        nc.vector.tensor_copy(out=qt16[:, h, :], in_=qt_ps[:])
                        qt_pss[h] = qt_ps

                    # S3: keys ssq (PE; Pool squares done head-wise) + rstd
                    # (ACT) + kts (DVE); ssk single-buffered 4-bank tile
                    for h in range(H):
                        ssqk_ps = ps_ssk.tile([128, K], f32, tag="ssk")
                        for c in range(K // 512):
                            nc.tensor.matmul(ssqk_ps[:, c * 512:(c + 1) * 512],
                                             w1m[:, h, :],
                                             sq16k[:, h, c * 512:(c + 1) * 512],
                                             start=True, stop=True)
                        rstd16 = rk_p.tile([128, K], bf16, tag="rk")
                        nc.scalar.activation(out=rstd16[:], in_=ssqk_ps[:],
                                             func=AF.Abs_reciprocal_sqrt,
                                             bias=eps_t[:], scale=1.0)
                        nc.vector.tensor_tensor(out=kts16[:, h, :],
                                                in0=kT16f[:, h, :],
                                                in1=rstd16[:], op=OP.mult)

                    # S4: q squares (Pool) + ssq matmuls into head-pairs
                    # (PE) + rsqrt per pair (ACT) + qhat (DVE)
                    sqq_all = sqq_p.tile([128, H, SC], bf16, tag="sqq")
                    for h in range(H):
                        nc.gpsimd.tensor_tensor(out=sqq_all[:, h, :],
                                                in0=qt16[:, h, :],
                                                in1=qt16[:, h, :], op=OP.mult)
                    rqs = {}
                    for hp in range(H // 2):
                        ssqq_ps = ps_sqq.tile([128, 2, SC], f32, tag="ssqq")
                        for i in range(2):
                            nc.tensor.matmul(ssqq_ps[:, i, :], ones16[:],
                                             sqq_all[:, 2 * hp + i, :],
                                             start=True, stop=True)
                        rqp = rq_p.tile([128, 2, SC], f32, tag="rq")
                        nc.scalar.activation(out=rqp[:], in_=ssqq_ps[:],
                                             func=AF.Abs_reciprocal_sqrt,
                                             bias=eps_t[:], scale=1.0)
                        rqs[hp] = rqp
                    for h in range(H):
                        sgn = -1.0 if h in neg_heads else 1.0
                        nc.vector.scalar_tensor_tensor(
                            out=qhat16[:, h, :], in0=qt16[:, h, :], scalar=sgn,
                            in1=rqs[h // 2][:, h % 2, :], op0=OP.mult, op1=OP.mult)

                # ---- Phase B (att pool double-buffered, exp 3-ktile batches)
                BATCHES = [(0, 3), (3, 3), (6, 3), (9, 3), (12, 3), (15, 1)]
                with tc.tile_pool(name="vload", bufs=3) as vload, \
                     tc.tile_pool(name="e_p", bufs=3) as e_p, \
                     tc.tile_pool(name="d1_p", bufs=2) as d1_p, \
                     tc.tile_pool(name="d2_p", bufs=2) as d2_p, \
                     tc.tile_pool(name="rec_p", bufs=2) as rec_p, \
                     tc.tile_pool(name="wo_p", bufs=1) as wo_p:
                    Wo_sb = wo_p.tile([128, H, D], bf16)
                    hstate = {}
                    P = {}

                    def emit_head_fwd(h):
                        ps_att, ps_y = P["att"], P["y"]
                        v_sb = vload.tile([128, KT, HD], bf16, tag="vf")
                        nc.gpsimd.dma_start(out=v_sb[:], in_=v16[:, h, :, :])
                        E16 = e_p.tile([128, KT, SC], bf16, tag="E")
                        # y matmuls of head h-1: all E values ready -> pure
                        # PE filler between this head's score batches
                        yjobs = []
                        if h >= 1:
                            Eprev, vprev = hstate[h - 1][0], hstate[h - 1][2]
                            yt_prev = ps_y.tile([128, SC], f32, tag="yt")
                            hstate[h - 1] = (Eprev, yt_prev, vprev)
                            yjobs = list(range(KT))
                        for t0, nt in BATCHES:
                            att_ps = ps_att.tile([128, 3, SC], f32, tag="att")
                            for tt in range(nt):
                                t = t0 + tt
                                nc.tensor.matmul(
                                    att_ps[:, tt, :],
                                    kts16[:, h, t * 128:(t + 1) * 128],
                                    qhat16[:, h, :], start=True, stop=True)
                            for _ in range(3):
                                if yjobs:
                                    t = yjobs.pop(0)
                                    nc.tensor.matmul(yt_prev[:], vprev[:, t, :],
                                                     Eprev[:, t, :],
                                                     start=(t == 0),
                                                     stop=(t == KT - 1))
                            nc.scalar.activation(
                                out=E16[:, t0:t0 + nt, :],
                                in_=att_ps[:, 0:nt, :], func=AF.Exp,
                                bias=0.0, scale=1.0)
                        while yjobs:
                            t = yjobs.pop(0)
                            nc.tensor.matmul(yt_prev[:], vprev[:, t, :],
                                             Eprev[:, t, :],
                                             start=(t == 0), stop=(t == KT - 1))
                        hstate[h] = (E16, None, v_sb)

                    def emit_head_norm(h):
                        E16, yt_ps, _v = hstate.pop(h)
                        d1 = d1_p.tile([128, 8, SC], bf16, tag="d1")
                        nc.vector.tensor_tensor(out=d1[:], in0=E16[:, 0:8, :],
                                                in1=E16[:, 8:16, :], op=OP.add)
                        d2 = d2_p.tile([128, 4, SC], bf16, tag="d2")
                        nc.vector.tensor_tensor(out=d2[:], in0=d1[:, 0:4, :],
                                                in1=d1[:, 4:8, :], op=OP.add)
                        d3 = d2_p.tile([128, 2, SC], bf16, tag="d3")
                        nc.vector.tensor_tensor(out=d3[:], in0=d2[:, 0:2, :],
                                                in1=d2[:, 2:4, :], op=OP.add)
                        d4 = rec_p.tile([128, SC], f32, tag="d4")
                        nc.vector.tensor_tensor(out=d4[:], in0=d3[:, 0, :],
                                                in1=d3[:, 1, :], op=OP.add)
                        den = rec_p.tile([128, SC], f32, tag="den")
                        nc.gpsimd.partition_all_reduce(den[:], d4[:], 128,
                                                       bass_isa.ReduceOp.add)
                        recd = rec_p.tile([128, SC], f32, tag="recd")
                        nc.vector.reciprocal_approx_fast(out=recd[:], in_=den[:])
                        nc.vector.tensor_tensor(out=ynorm[:, h, :], in0=yt_ps[:],
                                                in1=recd[:], op=OP.mult)

                    with tc.tile_pool(name="ps_att", bufs=2, space="PSUM") as ps_att, \
                         tc.tile_pool(name="ps_y", bufs=2, space="PSUM") as ps_y:
                        P["att"], P["y"] = ps_att, ps_y
                        for h in range(H):
                            emit_head_fwd(h)
                            if h == 6:
                                nc.sync.dma_start(out=Wo_sb[:], in_=Wo16[:])
                            if h >= 2:
                                emit_head_norm(h - 2)
                        # flush: y of head H-1, then norms of H-2, H-1
                        Eprev, _, vprev = hstate[H - 1]
                        yt_prev = ps_y.tile([128, SC], f32, tag="yt")
                        hstate[H - 1] = (Eprev, yt_prev, vprev)
                        for t in range(KT):
                            nc.tensor.matmul(yt_prev[:], vprev[:, t, :],
                                             Eprev[:, t, :],
                                             start=(t == 0), stop=(t == KT - 1))
                        emit_head_norm(H - 2)
                        emit_head_norm(H - 1)

                    # ---- Phase C
                    with tc.tile_pool(name="outsb", bufs=1) as outsb, \
                         tc.tile_pool(name="ps_out", bufs=2, space="PSUM") as ps_out:
                        out_all = outsb.tile([128, 4, 2, 512], bf16)
                        for si in range(SC // 128):
                            for oc in range(D // 512):
                                o_ps = ps_out.tile([128, 512], f32, tag="ops")
                                for h in range(H):
                                    nc.tensor.matmul(
                                        o_ps[:],
                                        ynorm[:, h, si * 128:(si + 1) * 128],
                                        Wo_sb[:, h, oc * 512:(oc + 1) * 512],
                                        start=(h == 0), stop=(h == H - 1))
                                nc.vector.tensor_copy(out=out_all[:, si, oc, :],
                                                      in_=o_ps[:])
                        out_view = out.rearrange("(si p) (oc c) -> p si oc c",
                                                 p=128, c=512)
                        nc.sync.dma_start(out=out_view, in_=out_all[:])

        if reps > 1:
            with tc.For_i(0, reps, 1):
                body()
        else:
            body()

    nc.compile()
    return nc


_CACHE = {}


def _get_nc(neg_heads, reps=1):
    key = (tuple(sorted(neg_heads)), reps)
    if key not in _CACHE:
        _CACHE[key] = build_nc(neg_heads, reps)
    return _CACHE[key]


def _make_in_maps(x, Wq, keys, values, attn_scale, Wo):
    bf16np = ml_dtypes.bfloat16
    x = np.asarray(x, dtype=np.float32).reshape(S, D)
    Wq = np.asarray(Wq, dtype=np.float32)
    Wo = np.asarray(Wo, dtype=np.float32)
    keys = np.asarray(keys, dtype=np.float32).reshape(K, H, HD)
    values = np.asarray(values, dtype=np.float32).reshape(K, H, HD)
    attn_scale = np.ascontiguousarray(np.asarray(attn_scale, dtype=np.float32))

    Wq16 = np.ascontiguousarray(
        Wq.reshape(MT, 128, D).transpose(1, 0, 2)).astype(bf16np)
    kT16 = np.ascontiguousarray(keys.transpose(2, 1, 0)).astype(bf16np)
    v16h = np.ascontiguousarray(
        values.reshape(KT, 128, H, HD).transpose(1, 2, 0, 3)).astype(bf16np)
    Wo16 = np.ascontiguousarray(
        Wo.reshape(H, 128, D).transpose(1, 0, 2)).astype(bf16np)

    in_maps = []
    for c in range(N_CORES):
        xc = x[c * SC:(c + 1) * SC, :]
        xT16c = np.ascontiguousarray(
            xc.T.reshape(MT, 128, SC).transpose(1, 0, 2)).astype(bf16np)
        in_maps.append({
            "xT16": xT16c, "Wq16": Wq16, "kT16": kT16, "v16": v16h,
            "Wo16": Wo16, "scale": attn_scale,
        })
    return in_maps


def kernel(x, Wq, keys, values, attn_scale, Wo):
    neg_heads = tuple(np.nonzero(np.asarray(attn_scale) < 0)[0].tolist())
    nc = _get_nc(neg_heads)
    in_maps = _make_in_maps(x, Wq, keys, values, attn_scale, Wo)
    res = run_bass_kernel_spmd(nc, in_maps, list(range(N_CORES)))
    out = np.concatenate([np.asarray(r["out"], dtype=np.float32)
                          for r in res.results], axis=0)
    return out.reshape(B, S, D)
